# revision 27
# baseline (speedup 1.0000x reference)
"""Causal BoW (running mean over T) Trainium2 kernel.

out[b, t, c] = sum_{s<=t} x[b, s, c] / (t+1)   for x of shape [32, 2048, 512] f32.

Sharding: batch B=32 across 8 NeuronCores (4 samples each), no cross-core comms.

Per-core algorithm (per sample [T=2048, C=512], 16 T-blocks of 128 rows):
  - Single-pass f32r matmuls: x is loaded into f32r tiles and streamed
    through the PE directly (1 cycle/column instead of 4 for f32). The
    ~11-bit mantissa truncation gives ~1e-4 relative output error, far
    inside the 2e-2 tolerance.
  - Block offsets: one accumulating matmul group with "step" selector
    weights (step_k[p, m] = 1 if m > k) produces off[m, c] = sum_{k<m} tot_k
    in one PSUM bank.
  - Offset injection: off2[j] = x[b, j*128, :] + off[j] is one [16,512] DVE
    add (the block-top rows are pre-gathered from DRAM by a tiny DMA), then
    four small HWDGE SBUF->SBUF DMAs overwrite row 0 of each block with
    off2 (DMA moves data across partitions freely; compute engines cannot).
    Since every column m of U128 includes row 0, the scan matmul broadcasts
    the offset to all 128 output rows for free. (A K=16 one-hot PE matmul
    can do the broadcast without any DMA, but at the HAM-throttled 1.2 GHz
    PE clock the 60 extra matmuls make the PE the bottleneck: measured
    113 us vs 110 us for this scheme.)
  - RING PLACEMENT of the scatters is the critical scheduling decision.
    HWDGE rings (SP=sync, Act=scalar) are FIFO at descriptor level and the
    tile scheduler orders each engine's stream by sim-readiness. Left to
    itself it queues each scatter c(b) behind all bulk loads (SP) or behind
    ~3 MB of store groups (Act); either serializes every scan window ~15 us
    behind its data (measured 121 us / 110 us). Here sample b+2's loads are
    EMITTED late (slot 14 of window b) behind a tc.no_sync_barrier() fence,
    pinning the SP ring to [L0, c0, L1, c1, L2, c2, L3, c3]: each scatter
    drains the moment its offsets are ready (~2 us ring stall, absorbed by
    the store backlog on the other ring), and scans(b) start right after
    load(b) lands.
  - Block scan: psum_j = U128^T.T @ x_j (U128 = upper-triangular ones).
  - Eviction: DVE copy with per-partition scale recip[p, j] = 1/(j*128+p+1)
    applied while moving PSUM -> SBUF.
  - x lives in quarter tiles [128, 4*512] (4 per sample): dependency
    tracking is tile-level, so quarter tiles let the first offset matmuls
    start ~3 us after launch instead of waiting a whole 4 MB sample load.
  - Software pipeline: sample b+1's offset matmuls are interleaved between
    sample b's scan matmuls (slots 5..12, 2 per slot; off group closes at
    slot 13 so the off2-add + scatter chain hides behind the last scans).
  - Input loads issue on the SP (sync) HW-DGE queue, output stores and
    constants on the Activation HW-DGE queue, so store issue never queues
    behind dependent load issue and both streams keep all 16 DMA engines
    fed. The last sample's output goes out in 2-block chunks to shorten the
    final drain.
"""

import numpy as np

import concourse.bass as bass
import concourse.bacc as bacc
import concourse.mybir as mybir
from concourse import tile
from concourse.bass_utils import run_bass_kernel_spmd

B, T, C = 32, 2048, 512
N_CORES = 8
BS = B // N_CORES          # samples per core
P = 128                    # partitions / T-block size
NBLK = T // P              # 16 blocks per sample
NQ = 2                     # sections per sample (halves)
NH = NBLK // NQ            # blocks per section (8)
F32 = mybir.dt.float32
F32R = mybir.dt.float32r

_cache = {}


def _build():
    nc = bacc.Bacc()
    x = nc.dram_tensor("x", [BS, T, C], F32R, kind="ExternalInput")
    u128 = nc.dram_tensor("u128", [P, P], F32R, kind="ExternalInput")
    stepm = nc.dram_tensor("stepm", [P, NBLK * NBLK], F32R, kind="ExternalInput")
    recip = nc.dram_tensor("recip", [P, NBLK], F32, kind="ExternalInput")
    y = nc.dram_tensor("y", [BS, T, C], F32, kind="ExternalOutput")

    HALF = NH * C

    with tile.TileContext(nc) as tc:
        with (
            tc.tile_pool(name="singles", bufs=1) as singles,
            tc.tile_pool(name="xp", bufs=8) as xpool,
            tc.tile_pool(name="op", bufs=4) as opool,
            tc.tile_pool(name="op2", bufs=8) as opool2,
            tc.tile_pool(name="off2p", bufs=2) as off2pool,
            tc.tile_pool(name="pblk", bufs=6, space="PSUM") as pblk,
            tc.tile_pool(name="poff", bufs=2, space="PSUM") as poff,
        ):
            u_t = singles.tile([P, P], F32R)
            nc.scalar.dma_start(out=u_t[:], in_=u128[:])
            step_t = singles.tile([P, NBLK * NBLK], F32R)
            nc.scalar.dma_start(out=step_t[:], in_=stepm[:])
            recip_t = singles.tile([P, NBLK], F32)
            nc.scalar.dma_start(out=recip_t[:], in_=recip[:])

            def load(b):
                xs = x[b].rearrange("(j p) c -> p j c", p=P)   # [128, 16, 512]
                xts = []
                for h in range(NQ):
                    xt = xpool.tile([P, HALF], F32R, tag="xt", name="xt")
                    xt3 = xt.rearrange("p (j c) -> p j c", c=C)
                    nc.sync.dma_start(out=xt3[:],
                                      in_=xs[:, h * NH:(h + 1) * NH, :])
                    xts.append(xt)
                return xts

            def off_mm(xts, offp_t, k):
                sel = step_t[:, k * NBLK:(k + 1) * NBLK]
                nc.tensor.matmul(
                    offp_t[:], sel,
                    xts[k // NH][:, (k % NH) * C:(k % NH + 1) * C],
                    start=(k == 0), stop=(k == NBLK - 1),
                )

            def off_finish(xts, offp_t, ring):
                # the step constant already folds in x[b, j*128, :] (its
                # partition-0 diagonal term), so offp IS off2; just evict
                # PSUM -> SBUF so the scatter DMA can read it
                off2 = off2pool.tile([NBLK, C], F32R, tag="off2")
                nc.vector.tensor_scalar_mul(off2[:], offp_t[:], 1.0)
                # overwrite row 0 of every block (partition 0 of each half)
                for h in range(NQ):
                    ring.dma_start(out=xts[h][0:1, :],
                                   in_=off2[h * NH:(h + 1) * NH, :])

            def scan_window(b, xts, nxt):
                ys = y[b].rearrange("(j p) c -> p j c", p=P)
                last = nxt is None
                if not last:
                    nxt_xts = nxt
                    offp_t = poff.tile([NBLK, C], F32, tag="offp")
                    # scatters for samples 0..1 ride the Act ring: their
                    # drain latency hides behind load-saturated engines
                    # (on the SP ring they would stall the load stream,
                    # measured +28 us). Samples 2..3's scatters ride the
                    # SP ring, which is EMPTY once the bulk loads drain
                    # (~73 us): they fire the moment their offsets are
                    # ready instead of waiting behind ~3 store groups
                    # (measured: scan window 3 started 93 us -> ~81 us).
                    c_ring = nc.scalar if b + 1 < 2 else nc.sync
                ng, gb = (8, 2) if last else (4, 4)
                for h in range(ng):
                    # the last window has 8 half-size store groups; with the
                    # shared 4-buf pool its back half stalls on store-complete
                    # buffer recycling (~840 ns/block pacing) — give it a
                    # dedicated 8-buf pool so evictions never wait
                    pool = opool2 if last else opool
                    ot = pool.tile([P, gb * C], F32,
                                   tag="ot2" if last else "ot")
                    for jj in range(gb):
                        j = h * gb + jj
                        pb = pblk.tile([P, C], F32)
                        nc.tensor.matmul(
                            pb[:], u_t[:],
                            xts[j // NH][:, (j % NH) * C:(j % NH + 1) * C],
                            start=True, stop=True)
                        if not last and 5 <= j < 13:
                            off_mm(nxt_xts, offp_t, 2 * (j - 5))
                            off_mm(nxt_xts, offp_t, 2 * (j - 5) + 1)
                        elif not last and j == 13:
                            off_finish(nxt_xts, offp_t, c_ring)
                        nc.vector.tensor_scalar_mul(
                            ot[:, jj * C:(jj + 1) * C], pb[:],
                            recip_t[:, j:j + 1]
                        )
                    ot3 = ot.rearrange("p (j c) -> p j c", c=C)
                    nc.scalar.dma_start(
                        out=ys[:, h * gb:(h + 1) * gb, :], in_=ot3[:]
                    )

            # prologue: all loads up front; sample 0's offsets + injection
            xts = [load(bb) for bb in range(BS)]
            offp0 = poff.tile([NBLK, C], F32, tag="offp")
            for k in range(NBLK):
                off_mm(xts[0], offp0, k)
            off_finish(xts[0], offp0, nc.scalar)

            for b in range(BS):
                nxt = xts[b + 1] if b + 1 < BS else None
                scan_window(b, xts[b], nxt)
    nc.finalize()
    return nc


def _consts():
    u = np.triu(np.ones((P, P), dtype=np.float32))
    step = np.zeros((P, NBLK * NBLK), dtype=np.float32)
    for k in range(NBLK):
        for m in range(NBLK):
            if m > k:
                step[:, k * NBLK + m] = 1.0
        # diagonal partition-0 term folds x[b, m*128, :] into off2[m], so
        # no separate block-top-row gather (xr) is needed
        step[0, k * NBLK + k] = 1.0
    recip = (1.0 / np.arange(1, T + 1, dtype=np.float32)).reshape(NBLK, P).T.copy()
    return u, step, recip


def run(x, trace=False):
    x = np.ascontiguousarray(np.asarray(x, dtype=np.float32))
    assert x.shape == (B, T, C), x.shape
    if "nc" not in _cache:
        _cache["nc"] = _build()
    nc = _cache["nc"]
    u, step, recip = _consts()
    in_maps = [
        {
            "x": np.ascontiguousarray(x[i * BS:(i + 1) * BS]),
            "u128": u,
            "stepm": step,
            "recip": recip,
        }
        for i in range(N_CORES)
    ]
    res = run_bass_kernel_spmd(nc, in_maps, list(range(N_CORES)), trace=trace)
    y = np.concatenate([res.results[i]["y"] for i in range(N_CORES)], axis=0)
    return y, res.exec_time_ns


def kernel(x):
    y, _ = run(x, trace=False)
    return y


# revision 35
# speedup vs baseline: 1.1217x; 1.1217x over previous
"""Causal BoW (running mean over T) Trainium2 kernel.

out[b, t, c] = sum_{s<=t} x[b, s, c] / (t+1)   for x of shape [32, 2048, 512] f32.

Sharding: batch B=32 across 8 NeuronCores (4 samples each), no cross-core comms.

Per-core algorithm (per sample [T=2048, C=512], 16 T-blocks of 128 rows):
  - Single-pass f32r matmuls: x is loaded into f32r tiles and streamed
    through the PE directly (1 cycle/column instead of 4 for f32). The
    ~11-bit mantissa truncation gives ~1e-4 relative output error, far
    inside the 2e-2 tolerance.
  - Block offsets: one accumulating matmul group with "step" selector
    weights (step_k[p, m] = 1 if m > k) produces off[m, c] = sum_{k<m} tot_k
    in one PSUM bank.
  - Offset injection: off2[j] = x[b, j*128, :] + off[j] is one [16,512] DVE
    add (the block-top rows are pre-gathered from DRAM by a tiny DMA), then
    four small HWDGE SBUF->SBUF DMAs overwrite row 0 of each block with
    off2 (DMA moves data across partitions freely; compute engines cannot).
    Since every column m of U128 includes row 0, the scan matmul broadcasts
    the offset to all 128 output rows for free. (A K=16 one-hot PE matmul
    can do the broadcast without any DMA, but at the HAM-throttled 1.2 GHz
    PE clock the 60 extra matmuls make the PE the bottleneck: measured
    113 us vs 110 us for this scheme.)
  - RING PLACEMENT of the scatters is the critical scheduling decision.
    HWDGE rings (SP=sync, Act=scalar) are FIFO at descriptor level and the
    tile scheduler orders each engine's stream by sim-readiness. Left to
    itself it queues each scatter c(b) behind all bulk loads (SP) or behind
    ~3 MB of store groups (Act); either serializes every scan window ~15 us
    behind its data (measured 121 us / 110 us). Here sample b+2's loads are
    EMITTED late (slot 14 of window b) behind a tc.no_sync_barrier() fence,
    pinning the SP ring to [L0, c0, L1, c1, L2, c2, L3, c3]: each scatter
    drains the moment its offsets are ready (~2 us ring stall, absorbed by
    the store backlog on the other ring), and scans(b) start right after
    load(b) lands.
  - Block scan: psum_j = U128^T.T @ x_j (U128 = upper-triangular ones).
  - Eviction: DVE copy with per-partition scale recip[p, j] = 1/(j*128+p+1)
    applied while moving PSUM -> SBUF.
  - x lives in quarter tiles [128, 4*512] (4 per sample): dependency
    tracking is tile-level, so quarter tiles let the first offset matmuls
    start ~3 us after launch instead of waiting a whole 4 MB sample load.
  - Software pipeline: sample b+1's offset matmuls are interleaved between
    sample b's scan matmuls (slots 5..12, 2 per slot; off group closes at
    slot 13 so the off2-add + scatter chain hides behind the last scans).
  - Input loads issue on the SP (sync) HW-DGE queue, output stores and
    constants on the Activation HW-DGE queue, so store issue never queues
    behind dependent load issue and both streams keep all 16 DMA engines
    fed. The last sample's output goes out in 2-block chunks to shorten the
    final drain.
"""

import numpy as np

import concourse.bass as bass
import concourse.bacc as bacc
import concourse.mybir as mybir
from concourse import tile
from concourse.bass_utils import run_bass_kernel_spmd

B, T, C = 32, 2048, 512
N_CORES = 8
BS = B // N_CORES          # samples per core
P = 128                    # partitions / T-block size
NBLK = T // P              # 16 blocks per sample
NQ = 4                     # quarters per sample
NH = NBLK // NQ            # blocks per quarter (4)
F32 = mybir.dt.float32
F32R = mybir.dt.float32r

_cache = {}


def _build():
    nc = bacc.Bacc()
    x = nc.dram_tensor("x", [BS, T, C], F32R, kind="ExternalInput")
    u128 = nc.dram_tensor("u128", [P, P], F32R, kind="ExternalInput")
    stepm = nc.dram_tensor("stepm", [P, NBLK * NBLK], F32R, kind="ExternalInput")
    recip = nc.dram_tensor("recip", [P, NBLK], F32, kind="ExternalInput")
    y = nc.dram_tensor("y", [BS, T, C], F32, kind="ExternalOutput")

    HALF = NH * C

    with tile.TileContext(nc) as tc:
        with (
            tc.tile_pool(name="singles", bufs=1) as singles,
            tc.tile_pool(name="xp", bufs=16) as xpool,
            tc.tile_pool(name="op", bufs=4) as opool,
            tc.tile_pool(name="op2", bufs=8) as opool2,
            tc.tile_pool(name="off2p", bufs=2) as off2pool,
            tc.tile_pool(name="pblk", bufs=6, space="PSUM") as pblk,
            tc.tile_pool(name="poff", bufs=2, space="PSUM") as poff,
        ):
            u_t = singles.tile([P, P], F32R)
            nc.scalar.dma_start(out=u_t[:], in_=u128[:])
            step_t = singles.tile([P, NBLK * NBLK], F32R)
            nc.scalar.dma_start(out=step_t[:], in_=stepm[:])
            recip_t = singles.tile([P, NBLK], F32)
            nc.scalar.dma_start(out=recip_t[:], in_=recip[:])

            def load(b):
                xs = x[b].rearrange("(j p) c -> p j c", p=P)   # [128, 16, 512]
                xts = []
                for h in range(NQ):
                    xt = xpool.tile([P, HALF], F32R, tag="xt", name="xt")
                    xt3 = xt.rearrange("p (j c) -> p j c", c=C)
                    nc.sync.dma_start(out=xt3[:],
                                      in_=xs[:, h * NH:(h + 1) * NH, :])
                    xts.append(xt)
                return xts

            def off_mm(xts, offp_t, k):
                sel = step_t[:, k * NBLK:(k + 1) * NBLK]
                nc.tensor.matmul(
                    offp_t[:], sel,
                    xts[k // NH][:, (k % NH) * C:(k % NH + 1) * C],
                    start=(k == 0), stop=(k == NBLK - 1),
                )

            def off_finish(xts, offp_t, ring):
                # the step constant folds in x[b, j*128, :] (partition-0
                # diagonal term), so offp IS off2; evict PSUM -> SBUF so
                # the scatter DMA can read it (DMA has no PSUM route)
                off2 = off2pool.tile([NBLK, C], F32R, tag="off2")
                nc.vector.tensor_scalar_mul(off2[:], offp_t[:], 1.0)
                # overwrite row 0 of every block (partition 0 of each quarter)
                for h in range(NQ):
                    ring.dma_start(out=xts[h][0:1, :],
                                   in_=off2[h * NH:(h + 1) * NH, :])

            def scan_window(b, xts, nxt):
                ys = y[b].rearrange("(j p) c -> p j c", p=P)
                last = nxt is None
                if not last:
                    nxt_xts = nxt
                    offp_t = poff.tile([NBLK, C], F32, tag="offp")
                    # scatters for samples 0..1 ride the Act ring: their
                    # drain latency hides behind load-saturated engines
                    # (on the SP ring they would stall the load stream,
                    # measured +28 us). Samples 2..3's scatters ride the
                    # SP ring, which is EMPTY once the bulk loads drain
                    # (~73 us): they fire the moment their offsets are
                    # ready instead of waiting behind ~3 store groups
                    # (measured: scan window 3 started 93 us -> ~81 us).
                    c_ring = nc.scalar if b + 1 < 2 else nc.sync
                ng, gb = (8, NH // 2) if last else (NQ, NH)
                for h in range(ng):
                    # the last window has 8 half-size store groups; with the
                    # shared 4-buf pool its back half stalls on store-complete
                    # buffer recycling (~840 ns/block pacing) — give it a
                    # dedicated 8-buf pool so evictions never wait
                    pool = opool2 if last else opool
                    ot = pool.tile([P, gb * C], F32,
                                   tag="ot2" if last else "ot")
                    for jj in range(gb):
                        j = h * gb + jj
                        pb = pblk.tile([P, C], F32)
                        nc.tensor.matmul(
                            pb[:], u_t[:],
                            xts[j // NH][:, (j % NH) * C:(j % NH + 1) * C],
                            start=True, stop=True)
                        if not last and 5 <= j < 13:
                            off_mm(nxt_xts, offp_t, 2 * (j - 5))
                            off_mm(nxt_xts, offp_t, 2 * (j - 5) + 1)
                        elif not last and j == 13:
                            off_finish(nxt_xts, offp_t, c_ring)
                        nc.vector.tensor_scalar_mul(
                            ot[:, jj * C:(jj + 1) * C], pb[:],
                            recip_t[:, j:j + 1]
                        )
                    ot3 = ot.rearrange("p (j c) -> p j c", c=C)
                    nc.scalar.dma_start(
                        out=ys[:, h * gb:(h + 1) * gb, :], in_=ot3[:]
                    )

            # prologue: all loads up front; sample 0's offsets + injection
            xts = [load(bb) for bb in range(BS)]
            offp0 = poff.tile([NBLK, C], F32, tag="offp")
            for k in range(NBLK):
                off_mm(xts[0], offp0, k)
            off_finish(xts[0], offp0, nc.scalar)

            for b in range(BS):
                nxt = xts[b + 1] if b + 1 < BS else None
                scan_window(b, xts[b], nxt)
    nc.finalize()
    return nc


def _consts():
    u = np.triu(np.ones((P, P), dtype=np.float32))
    step = np.zeros((P, NBLK * NBLK), dtype=np.float32)
    for k in range(NBLK):
        for m in range(NBLK):
            if m > k:
                step[:, k * NBLK + m] = 1.0
        # diagonal partition-0 term folds x[b, k*128, :] into off2[k], so
        # no separate block-top-row gather (xr) is needed
        step[0, k * NBLK + k] = 1.0
    recip = (1.0 / np.arange(1, T + 1, dtype=np.float32)).reshape(NBLK, P).T.copy()
    return u, step, recip


def run(x, trace=False):
    x = np.ascontiguousarray(np.asarray(x, dtype=np.float32))
    assert x.shape == (B, T, C), x.shape
    if "nc" not in _cache:
        _cache["nc"] = _build()
    nc = _cache["nc"]
    u, step, recip = _consts()
    in_maps = [
        {
            "x": np.ascontiguousarray(x[i * BS:(i + 1) * BS]),
            "u128": u,
            "stepm": step,
            "recip": recip,
        }
        for i in range(N_CORES)
    ]
    res = run_bass_kernel_spmd(nc, in_maps, list(range(N_CORES)), trace=trace)
    y = np.concatenate([res.results[i]["y"] for i in range(N_CORES)], axis=0)
    return y, res.exec_time_ns


def kernel(x):
    y, _ = run(x, trace=False)
    return y
